# revision 4
# baseline (speedup 1.0000x reference)
"""ArcFace-style margin softmax CE loss on 8 Trainium2 cores.

Math: the reference is mean softmax-CE over logits = 64*clip(cos_theta)
with the label column replaced by 64*(ct*cos(m) - sqrt(1-ct^2)*sin(m)).
Since cos_theta lives in [0,1), every exponent 64*x - 64 is <= 0, so a
fixed offset of 64 replaces the per-row max of the log-sum-exp.  The
device then only needs per-row sums of exp(64*x - 64) over the
2048x50000 matrix — a pure streaming, memory-bound reduction.  The
label-column swap (one element per row) and the final mean are O(B)
and are done on the host in f64.

Sharding: data-parallel over rows, 256 rows per core (contiguous
slices of the input, zero host-side copies, no cross-core combine).

Kernel (per core, raw Bass — one semaphore wait per instruction, which
is all this walrus build's codegen accepts):
  sync  : stream 32 chunks [128 x 3125] HBM->SBUF, NBUF-deep rotation
  scalar: ACTIVATE Exp(64x-64) with accum_out -> per-chunk row-sums
  vector: two final 16-column reduces -> [128, 2]
Per-slot DMA semaphores make the ACT wait race-free (slot reuse is
serialized through the ACT completions themselves).
"""

import numpy as np

import concourse.bass as bass
import concourse.mybir as mybir
from concourse.bass_utils import run_bass_kernel_spmd

B, C = 2048, 50000
N_CORES = 8
RPC = B // N_CORES          # 256 rows per core
P = 128                     # SBUF partitions
ROW_TILES = RPC // P        # 2
SCALE = 64.0
EPS = 1e-7
CHUNK = 3125                # free-dim chunk; 16 chunks per row tile
N_CHUNKS = C // CHUNK       # 16
TOTAL = ROW_TILES * N_CHUNKS  # 32 chunks per core
NBUF = 8                    # input buffer rotation depth

_NC = None                  # cached Bass module (compiled once per process)
LAST_RESULTS = None         # BassKernelResults of the most recent run


def _build():
    nc = bass.Bass()
    # activation() lowers a float bias to a const AP; -64.0 isn't in the
    # built-in const database, so register it the same way Bass init does.
    cneg = nc.alloc_sbuf_tensor("const-float32-neg64", [P, 1], mybir.dt.float32)
    nc.gpsimd.memset(cneg.ap(), -SCALE)
    nc.const_aps.aps[(mybir.dt.float32, -SCALE)] = cneg.ap()
    nc.all_engine_barrier()

    x = nc.dram_tensor("x", [RPC, C], mybir.dt.float32, kind="ExternalInput")
    s = nc.dram_tensor("s", [P, ROW_TILES], mybir.dt.float32, kind="ExternalOutput")

    bufs = [
        nc.alloc_sbuf_tensor(f"buf{b}", [P, CHUNK], mybir.dt.float32)
        for b in range(NBUF)
    ]
    scratch = [
        nc.alloc_sbuf_tensor(f"scr{k}", [P, CHUNK], mybir.dt.float32)
        for k in range(2)
    ]
    partials = nc.alloc_sbuf_tensor("partials", [P, TOTAL], mybir.dt.float32)
    srow = nc.alloc_sbuf_tensor("srow", [P, ROW_TILES], mybir.dt.float32)

    def chunk_src(i):
        rt, ci = divmod(i, N_CHUNKS)
        return x[rt * P:(rt + 1) * P, ci * CHUNK:(ci + 1) * CHUNK]

    with (
        nc.semaphore("sem_act") as sem_act,
        nc.semaphore("sem_red") as sem_red,
        nc.semaphore("sem_out") as sem_out,
    ):
        sem_buf = []
        import contextlib
        with contextlib.ExitStack() as st:
            for b_ in range(NBUF):
                sem_buf.append(st.enter_context(nc.semaphore(f"sem_buf{b_}")))

            with nc.Block() as block:

                @block.sync
                def _(sync):
                    for i in range(TOTAL):
                        b = i % NBUF
                        if i >= NBUF:
                            # slot reuse: ACT #(i-NBUF) has consumed bufs[b]
                            sync.wait_ge(sem_act, i - NBUF + 1)
                        sync.dma_start(
                            out=bufs[b].ap(), in_=chunk_src(i)
                        ).then_inc(sem_buf[b], 16)
                    sync.wait_ge(sem_red, ROW_TILES)
                    sync.dma_start(out=s[:, :], in_=srow.ap()).then_inc(sem_out, 16)

                @block.scalar
                def _(scalar):
                    for i in range(TOTAL):
                        b = i % NBUF
                        # the (i//NBUF + 1)-th DMA into this slot is done;
                        # slot DMAs are serialized by the ACT chain itself,
                        # so this per-slot count is race-free.
                        scalar.wait_ge(sem_buf[b], 16 * (i // NBUF + 1))
                        scalar.activation(
                            scratch[i % 2].ap(),
                            bufs[b].ap(),
                            mybir.ActivationFunctionType.Exp,
                            bias=-SCALE,
                            scale=SCALE,
                            accum_out=partials.ap()[:, i:i + 1],
                        ).then_inc(sem_act, 1)

                @block.vector
                def _(vector):
                    for rt in range(ROW_TILES):
                        vector.wait_ge(sem_act, (rt + 1) * N_CHUNKS)
                        vector.reduce_sum(
                            srow.ap()[:, rt:rt + 1],
                            partials.ap()[:, rt * N_CHUNKS:(rt + 1) * N_CHUNKS],
                            axis=mybir.AxisListType.X,
                        ).then_inc(sem_red, 1)

    return nc


def kernel(cos_theta, labels, margins):
    global _NC, LAST_RESULTS
    ct = np.ascontiguousarray(np.asarray(cos_theta, dtype=np.float32))
    lab = np.asarray(labels).astype(np.int64)
    mg = np.asarray(margins, dtype=np.float64)
    assert ct.shape == (B, C)

    if _NC is None:
        _NC = _build()

    in_maps = [{"x": ct[i * RPC:(i + 1) * RPC]} for i in range(N_CORES)]
    LAST_RESULTS = run_bass_kernel_spmd(_NC, in_maps, list(range(N_CORES)))
    # s[p, rt] holds the row-sum of global row  core*RPC + rt*P + p
    S_dev = np.concatenate(
        [LAST_RESULTS.results[i]["s"].T.reshape(-1) for i in range(N_CORES)]
    ).astype(np.float64)

    # Host correction: swap the label column's contribution, O(B) work.
    rows = np.arange(B)
    ct_l_raw = ct[rows, lab].astype(np.float64)
    ct_l = np.clip(ct_l_raw, -1.0 + EPS, 1.0 - EPS)
    m = mg[lab]
    target = ct_l * np.cos(m) - np.sqrt(1.0 - ct_l * ct_l) * np.sin(m)
    z_new = SCALE * target
    S_corr = S_dev - np.exp(SCALE * ct_l_raw - SCALE) + np.exp(z_new - SCALE)
    loss_i = (SCALE + np.log(S_corr)) - z_new
    return np.array(loss_i.mean(), dtype=np.float32)


# revision 6
# speedup vs baseline: 1.0056x; 1.0056x over previous
"""ArcFace-style margin softmax CE loss on 8 Trainium2 cores.

Math: the reference is mean softmax-CE over logits = 64*clip(cos_theta)
with the label column replaced by 64*(ct*cos(m) - sqrt(1-ct^2)*sin(m)).
Since cos_theta lives in [0,1), every exponent 64*x - 64 is <= 0, so a
fixed offset of 64 replaces the per-row max of the log-sum-exp.  The
device then only needs per-row sums of exp(64*x - 64) over the
2048x50000 matrix — a pure streaming, memory-bound reduction.  The
label-column swap (one element per row) and the final mean are O(B)
and are done on the host in f64.

Sharding: data-parallel over rows, 256 rows per core (contiguous
slices of the input, zero host-side copies, no cross-core combine).

Kernel (per core, raw Bass — one semaphore wait per instruction, which
is all this walrus build's codegen accepts):
  sync  : stream 32 chunks [128 x w] HBM->SBUF, NBUF-deep rotation,
          then two tiny result stores
  scalar: ACTIVATE Exp(64x-64) with accum_out -> per-chunk row-sums
  vector: final reduces of the per-chunk partial columns
The last two chunks of the stream are small (1900/500 cols) so the
final exposed ACT after the last DMA is ~0.7us instead of ~2.9us, and
row-tile 1's 15 early partial columns are reduced before its last ACT
lands, leaving only a 2-column combine on the critical path.
Per-slot DMA semaphores make the ACT wait race-free (slot reuse is
serialized through the ACT completions themselves).
"""

import contextlib

import numpy as np

import concourse.bass as bass
import concourse.mybir as mybir
from concourse.bass_utils import run_bass_kernel_spmd

B, C = 2048, 50000
N_CORES = 8
RPC = B // N_CORES          # 256 rows per core
P = 128                     # SBUF partitions
ROW_TILES = RPC // P        # 2
SCALE = 64.0
EPS = 1e-7
NBUF = 8                    # input buffer rotation depth

# chunk widths per row tile; rt1 tapers so the tail ACT is short
W0 = [3125] * 16                      # row tile 0: uniform
W1 = [3400] * 14 + [1900, 500]        # row tile 1: 47600 + 2400 = 50000
assert sum(W0) == C and sum(W1) == C
WMAX = max(W0 + W1)

_NC = None                  # cached Bass module (compiled once per process)
LAST_RESULTS = None         # BassKernelResults of the most recent run


def _chunk_table():
    """[(row_tile, col_start, width)] in stream order."""
    out = []
    for rt, ws in ((0, W0), (1, W1)):
        col = 0
        for w in ws:
            out.append((rt, col, w))
            col += w
    return out


CHUNKS = _chunk_table()
TOTAL = len(CHUNKS)         # 32
N0 = len(W0)                # ACTs belonging to row tile 0


def _build():
    nc = bass.Bass()
    # activation() lowers a float bias to a const AP; -64.0 isn't in the
    # built-in const database, so register it the same way Bass init does
    # (but guard the first ACT with a semaphore instead of a full barrier
    # so the DMA stream starts immediately).
    cneg = nc.alloc_sbuf_tensor("const-float32-neg64", [P, 1], mybir.dt.float32)
    nc.const_aps.aps[(mybir.dt.float32, -SCALE)] = cneg.ap()

    x = nc.dram_tensor("x", [RPC, C], mybir.dt.float32, kind="ExternalInput")
    s = nc.dram_tensor("s", [ROW_TILES, P], mybir.dt.float32, kind="ExternalOutput")

    bufs = [
        nc.alloc_sbuf_tensor(f"buf{b}", [P, WMAX], mybir.dt.float32)
        for b in range(NBUF)
    ]
    scratch = [
        nc.alloc_sbuf_tensor(f"scr{k}", [P, WMAX], mybir.dt.float32)
        for k in range(2)
    ]
    partials = nc.alloc_sbuf_tensor("partials", [P, TOTAL], mybir.dt.float32)
    srow = nc.alloc_sbuf_tensor("srow", [P, ROW_TILES], mybir.dt.float32)
    srowp = nc.alloc_sbuf_tensor("srowp", [P, 1], mybir.dt.float32)

    def chunk_src(i):
        rt, col, w = CHUNKS[i]
        return x[rt * P:(rt + 1) * P, col:col + w]

    with (
        nc.semaphore("sem_const") as sem_const,
        nc.semaphore("sem_act") as sem_act,
        nc.semaphore("sem_red") as sem_red,
        nc.semaphore("sem_out") as sem_out,
        contextlib.ExitStack() as st,
    ):
        sem_buf = [st.enter_context(nc.semaphore(f"sem_buf{b_}"))
                   for b_ in range(NBUF)]

        with nc.Block() as block:

            @block.gpsimd
            def _(gpsimd):
                gpsimd.memset(cneg.ap(), -SCALE).then_inc(sem_const, 1)

            @block.sync
            def _(sync):
                for i in range(TOTAL):
                    b = i % NBUF
                    if i >= NBUF:
                        # slot reuse: ACT #(i-NBUF) has consumed bufs[b]
                        sync.wait_ge(sem_act, i - NBUF + 1)
                    sync.dma_start(
                        out=bufs[b].ap()[:, :CHUNKS[i][2]], in_=chunk_src(i)
                    ).then_inc(sem_buf[b], 16)
                for rt in range(ROW_TILES):
                    sync.wait_ge(sem_red, rt + 1)
                    sync.dma_start(
                        out=s[rt, :], in_=srow.ap()[:, rt:rt + 1]
                    ).then_inc(sem_out, 16)

            @block.scalar
            def _(scalar):
                scalar.wait_ge(sem_const, 1)
                for i in range(TOTAL):
                    b = i % NBUF
                    w = CHUNKS[i][2]
                    # the (i//NBUF + 1)-th DMA into this slot is done;
                    # slot DMAs are serialized by the ACT chain itself,
                    # so this per-slot count is race-free.
                    scalar.wait_ge(sem_buf[b], 16 * (i // NBUF + 1))
                    scalar.activation(
                        scratch[i % 2].ap()[:, :w],
                        bufs[b].ap()[:, :w],
                        mybir.ActivationFunctionType.Exp,
                        bias=-SCALE,
                        scale=SCALE,
                        accum_out=partials.ap()[:, i:i + 1],
                    ).then_inc(sem_act, 1)

            @block.vector
            def _(vector):
                # row tile 0: one 16-column reduce, hidden under the stream
                vector.wait_ge(sem_act, N0)
                vector.reduce_sum(
                    srow.ap()[:, 0:1], partials.ap()[:, 0:N0],
                    axis=mybir.AxisListType.X,
                ).then_inc(sem_red, 1)
                # row tile 1: reduce all but the last column early, then a
                # 2-column combine once the final (short) ACT lands
                vector.wait_ge(sem_act, TOTAL - 1)
                vector.reduce_sum(
                    srowp.ap()[:, 0:1], partials.ap()[:, N0:TOTAL - 1],
                    axis=mybir.AxisListType.X,
                )
                vector.wait_ge(sem_act, TOTAL)
                vector.tensor_add(
                    srow.ap()[:, 1:2], srowp.ap()[:, 0:1],
                    partials.ap()[:, TOTAL - 1:TOTAL],
                ).then_inc(sem_red, 1)

    return nc


def kernel(cos_theta, labels, margins):
    global _NC, LAST_RESULTS
    ct = np.ascontiguousarray(np.asarray(cos_theta, dtype=np.float32))
    lab = np.asarray(labels).astype(np.int64)
    mg = np.asarray(margins, dtype=np.float64)
    assert ct.shape == (B, C)

    if _NC is None:
        _NC = _build()

    in_maps = [{"x": ct[i * RPC:(i + 1) * RPC]} for i in range(N_CORES)]
    LAST_RESULTS = run_bass_kernel_spmd(_NC, in_maps, list(range(N_CORES)))
    # s[rt, p] holds the row-sum of global row  core*RPC + rt*P + p
    S_dev = np.concatenate(
        [LAST_RESULTS.results[i]["s"].reshape(-1) for i in range(N_CORES)]
    ).astype(np.float64)

    # Host correction: swap the label column's contribution, O(B) work.
    rows = np.arange(B)
    ct_l_raw = ct[rows, lab].astype(np.float64)
    ct_l = np.clip(ct_l_raw, -1.0 + EPS, 1.0 - EPS)
    m = mg[lab]
    target = ct_l * np.cos(m) - np.sqrt(1.0 - ct_l * ct_l) * np.sin(m)
    z_new = SCALE * target
    S_corr = S_dev - np.exp(SCALE * ct_l_raw - SCALE) + np.exp(z_new - SCALE)
    loss_i = (SCALE + np.log(S_corr)) - z_new
    return np.array(loss_i.mean(), dtype=np.float32)


# revision 7
# speedup vs baseline: 1.0063x; 1.0007x over previous
"""ArcFace-style margin softmax CE loss on 8 Trainium2 cores.

Math: the reference is mean softmax-CE over logits = 64*clip(cos_theta)
with the label column replaced by 64*(ct*cos(m) - sqrt(1-ct^2)*sin(m)).
Since cos_theta lives in [0,1), every exponent 64*x - 64 is <= 0, so a
fixed offset of 64 replaces the per-row max of the log-sum-exp.  The
device then only needs per-row sums of exp(64*x - 64) over the
2048x50000 matrix — a pure streaming, memory-bound reduction.  The
label-column swap (one element per row) and the final mean are O(B)
and are done on the host in f64.

Sharding: data-parallel over rows, 256 rows per core (contiguous
slices of the input, zero host-side copies, no cross-core combine).

Kernel (per core, raw Bass — one semaphore wait per instruction, which
is all this walrus build's codegen accepts):
  sync  : stream 32 chunks [128 x w] HBM->SBUF, NBUF-deep rotation,
          then two tiny result stores
  scalar: ACTIVATE Exp(64x-64) with accum_out -> per-chunk row-sums
The per-chunk partial sums [128 x 32] are stored straight to DRAM
(16KB) and the final 16-column adds run on the host in f64 — no DVE
stage on the critical path.  The last two chunks of the stream are
small (1900/500 cols) so the final exposed ACT after the last DMA is
~0.7us instead of ~2.9us.
Per-slot DMA semaphores make the ACT wait race-free (slot reuse is
serialized through the ACT completions themselves).
"""

import contextlib

import numpy as np

import concourse.bass as bass
import concourse.mybir as mybir
from concourse.bass_utils import run_bass_kernel_spmd

B, C = 2048, 50000
N_CORES = 8
RPC = B // N_CORES          # 256 rows per core
P = 128                     # SBUF partitions
ROW_TILES = RPC // P        # 2
SCALE = 64.0
EPS = 1e-7
NBUF = 8                    # input buffer rotation depth

# chunk widths per row tile; rt1 tapers so the tail ACT is short
W0 = [3125] * 16                      # row tile 0: uniform
W1 = [3400] * 14 + [1900, 500]        # row tile 1: 47600 + 2400 = 50000
assert sum(W0) == C and sum(W1) == C
WMAX = max(W0 + W1)

_NC = None                  # cached Bass module (compiled once per process)
LAST_RESULTS = None         # BassKernelResults of the most recent run


def _chunk_table():
    """[(row_tile, col_start, width)] in stream order."""
    out = []
    for rt, ws in ((0, W0), (1, W1)):
        col = 0
        for w in ws:
            out.append((rt, col, w))
            col += w
    return out


CHUNKS = _chunk_table()
TOTAL = len(CHUNKS)         # 32
N0 = len(W0)                # ACTs belonging to row tile 0


def _build():
    nc = bass.Bass()
    # activation() lowers a float bias to a const AP; -64.0 isn't in the
    # built-in const database, so register it the same way Bass init does
    # (but guard the first ACT with a semaphore instead of a full barrier
    # so the DMA stream starts immediately).
    cneg = nc.alloc_sbuf_tensor("const-float32-neg64", [P, 1], mybir.dt.float32)
    nc.const_aps.aps[(mybir.dt.float32, -SCALE)] = cneg.ap()

    x = nc.dram_tensor("x", [RPC, C], mybir.dt.float32, kind="ExternalInput")
    s = nc.dram_tensor("s", [P, TOTAL], mybir.dt.float32, kind="ExternalOutput")

    bufs = [
        nc.alloc_sbuf_tensor(f"buf{b}", [P, WMAX], mybir.dt.float32)
        for b in range(NBUF)
    ]
    scratch = [
        nc.alloc_sbuf_tensor(f"scr{k}", [P, WMAX], mybir.dt.float32)
        for k in range(2)
    ]
    partials = nc.alloc_sbuf_tensor("partials", [P, TOTAL], mybir.dt.float32)

    def chunk_src(i):
        rt, col, w = CHUNKS[i]
        return x[rt * P:(rt + 1) * P, col:col + w]

    with (
        nc.semaphore("sem_const") as sem_const,
        nc.semaphore("sem_act") as sem_act,
        nc.semaphore("sem_out") as sem_out,
        contextlib.ExitStack() as st,
    ):
        sem_buf = [st.enter_context(nc.semaphore(f"sem_buf{b_}"))
                   for b_ in range(NBUF)]

        with nc.Block() as block:

            @block.gpsimd
            def _(gpsimd):
                gpsimd.memset(cneg.ap(), -SCALE).then_inc(sem_const, 1)

            @block.sync
            def _(sync):
                for i in range(TOTAL):
                    b = i % NBUF
                    if i >= NBUF:
                        # slot reuse: ACT #(i-NBUF) has consumed bufs[b]
                        sync.wait_ge(sem_act, i - NBUF + 1)
                    sync.dma_start(
                        out=bufs[b].ap()[:, :CHUNKS[i][2]], in_=chunk_src(i)
                    ).then_inc(sem_buf[b], 16)
                sync.wait_ge(sem_act, TOTAL)
                sync.dma_start(out=s[:, :], in_=partials.ap()
                               ).then_inc(sem_out, 16)

            @block.scalar
            def _(scalar):
                scalar.wait_ge(sem_const, 1)
                for i in range(TOTAL):
                    b = i % NBUF
                    w = CHUNKS[i][2]
                    # the (i//NBUF + 1)-th DMA into this slot is done;
                    # slot DMAs are serialized by the ACT chain itself,
                    # so this per-slot count is race-free.
                    scalar.wait_ge(sem_buf[b], 16 * (i // NBUF + 1))
                    scalar.activation(
                        scratch[i % 2].ap()[:, :w],
                        bufs[b].ap()[:, :w],
                        mybir.ActivationFunctionType.Exp,
                        bias=-SCALE,
                        scale=SCALE,
                        accum_out=partials.ap()[:, i:i + 1],
                    ).then_inc(sem_act, 1)


    return nc


def kernel(cos_theta, labels, margins):
    global _NC, LAST_RESULTS
    ct = np.ascontiguousarray(np.asarray(cos_theta, dtype=np.float32))
    lab = np.asarray(labels).astype(np.int64)
    mg = np.asarray(margins, dtype=np.float64)
    assert ct.shape == (B, C)

    if _NC is None:
        _NC = _build()

    in_maps = [{"x": ct[i * RPC:(i + 1) * RPC]} for i in range(N_CORES)]
    LAST_RESULTS = run_bass_kernel_spmd(_NC, in_maps, list(range(N_CORES)))
    # s[p, i] is chunk i's partial row-sum for global row
    # core*RPC + rt(i)*P + p; finish the reduction here in f64
    S_parts = []
    for i in range(N_CORES):
        ps = LAST_RESULTS.results[i]["s"].astype(np.float64)  # [P, TOTAL]
        S_parts.append(ps[:, :N0].sum(axis=1))        # rows rt0
        S_parts.append(ps[:, N0:].sum(axis=1))        # rows rt1
    S_dev = np.concatenate(S_parts)

    # Host correction: swap the label column's contribution, O(B) work.
    rows = np.arange(B)
    ct_l_raw = ct[rows, lab].astype(np.float64)
    ct_l = np.clip(ct_l_raw, -1.0 + EPS, 1.0 - EPS)
    m = mg[lab]
    target = ct_l * np.cos(m) - np.sqrt(1.0 - ct_l * ct_l) * np.sin(m)
    z_new = SCALE * target
    S_corr = S_dev - np.exp(SCALE * ct_l_raw - SCALE) + np.exp(z_new - SCALE)
    loss_i = (SCALE + np.log(S_corr)) - z_new
    return np.array(loss_i.mean(), dtype=np.float32)
